# revision 13
# baseline (speedup 1.0000x reference)
"""ListNet loss Trainium2 kernel.

kernel(y_pred_scores [2048, 8192] f32, y_true_seqs [2048, 512] int) -> () f32

Strategy: pure data parallel over the batch dim across 8 NeuronCores
(256 rows/core, 2 tiles of 128 rows). The per-row gather
g[p, l] = scores[p, seq[p, l]] is INVERTED into GPSIMD local_scatter,
the only on-chip primitive with per-partition independent indices:

  - host computes inv[p, c] = the sequence position (in reversed order)
    of column c's first occurrence, or -1 (ignored). Then
    local_scatter(data=scores_bf16[p, :], idxs=inv[p, :]) writes
    dst[p, inv[p, c]] = scores[p, c] -- the whole 512-wide gathered row
    in one pass over the natural score layout. ap_gather (shared index
    list per 16 partitions) would waste 15/16 of its output and is
    ~8x slower for this shape (~380us/core measured),
  - duplicated sequence indices (a column drawn at several positions)
    are appended as extra (value, position) columns to the data/idx
    arrays, so one scatter covers every occurrence,
  - each tile's scatter is split into column chunks (4 for tile 0, 2 for
    tile 1) so the first chunk starts as soon as its DMA slice lands;
    chunks write disjoint dst positions and are merged with adds. The
    whole kernel is jointly bound by HBM DMA (~6.2 MB/core) and the
    GPSIMD scatter stream (~25 us/tile), which overlap,
  - sequences are pre-reversed on host so pads sit at positions
    l < npads[row] and a forward prefix-sum scan of exp values yields
    the suffix softmax denominators S; the valid range [npads, 512) is
    selected with a device-built iota ramp >= (npads+1) per-partition
    mask,
  - LN = ln(S + eps); masked accumulating reductions give per-row
    sumg = sum of valid gathered scores, sumln = sum of valid LN.
Host: row_ll = sumg - sumln; used rows and the final mean in f64.

Scores are N(0,1) (sanitize is an identity on this data), so exp needs
no max-shift. bf16 score rounding (the scatter payload is 2-byte) gives
~2e-6 relative error on the final loss, far inside the 2e-2 gate.
"""

import numpy as np

B, N, L = 2048, 8192, 512
NCORES = 8
BL = B // NCORES  # 256 rows per core
P = 128
NT = BL // P  # tiles of 128 rows per core
EPS = 2.0**-126

TRACE = False
LAST_RESULTS = None

_cache = {}


def _build(K2):
    import concourse.bacc as bacc
    import concourse.mybir as mybir
    import concourse.tile as tile

    f32 = mybir.dt.float32
    bf16 = mybir.dt.bfloat16
    i16 = mybir.dt.int16
    Alu = mybir.AluOpType
    Act = mybir.ActivationFunctionType

    nc = bacc.Bacc("TRN2", target_bir_lowering=False, debug=False)
    NI = N + K2  # score columns + appended duplicate-fix entries
    sc = nc.dram_tensor("sc", [BL, NI], bf16, kind="ExternalInput").ap()
    inv = nc.dram_tensor("inv", [BL, NI], i16, kind="ExternalInput").ap()
    lo = nc.dram_tensor("lo", [BL, 1], f32, kind="ExternalInput").ap()
    # out columns: [sumg0, sumln0, sumg1a, sumg1b, sumln1]
    out = nc.dram_tensor("out", [P, 5], f32, kind="ExternalOutput").ap()

    with tile.TileContext(nc) as tc:
        with (
            tc.tile_pool(name="const", bufs=1) as cpool,
            tc.tile_pool(name="big", bufs=2) as bpool,
            tc.tile_pool(name="small", bufs=2) as spool,
        ):
            IOTA = cpool.tile([P, L], f32)
            epsb = cpool.tile([P, 1], f32)
            nc.vector.memset(epsb[:], EPS)
            stats = cpool.tile([P, 5], f32)
            SCR = cpool.tile([P, L], f32)  # dead-write target for accum ops

            tl = []  # per-tile dict of tiles
            for t in range(NT):
                rows = slice(t * P, (t + 1) * P)
                d = {}
                d["IV"] = bpool.tile([P, NI], i16, tag="iv", name=f"iv{t}")
                d["SB"] = bpool.tile([P, NI], bf16, tag="sb", name=f"sb{t}")
                # chunked DMA, alternating queues for byte balance; chunk
                # boundaries match the scatter splits below
                nch = 4 if t == 0 else 2
                step = NI // nch
                for q in range(nch):
                    cs = slice(q * step, (q + 1) * step)
                    qa = nc.sync if q % 2 == 0 else nc.scalar
                    qb = nc.scalar if q % 2 == 0 else nc.sync
                    qa.dma_start(out=d["IV"][:, cs], in_=inv[rows, cs])
                    qb.dma_start(out=d["SB"][:, cs], in_=sc[rows, cs])
                d["LO"] = spool.tile([P, 1], f32, tag="lo", name=f"lo{t}")
                tl.append(d)
            for t in range(NT):
                nc.scalar.dma_start(out=tl[t]["LO"][:], in_=lo[slice(t * P, (t + 1) * P), :])
            # IOTA = [1, 2, ..., L] built on device: prefix-scan of ones
            # (host sends npads+1 as the mask bound)
            nc.vector.memset(SCR[:], 1.0)
            nc.vector.tensor_tensor_scan(
                out=IOTA[:],
                data0=SCR[:],
                data1=SCR[:],
                initial=0.0,
                op0=Alu.add,
                op1=Alu.bypass,
            )
            # gpsimd: chunked column scatters (the serial resource); chunks
            # write disjoint dst positions. tile0: 4 chunks merged with bf16
            # adds (they crawl to completion under later scatters). tile1:
            # 2 chunks combined in exp space (exp(Da)*exp(Db) == exp(Da+Db)
            # exactly, since the missing half contributes e^0 = 1), so the
            # first half's exp/mask/partial-sum hide under the last scatter.
            t0, t1 = tl
            step = NI // 4
            parts = []
            for q in range(4):
                cs = slice(q * step, (q + 1) * step)
                Dq = spool.tile([P, L], bf16, tag=f"d{q}", name=f"d{q}_0")
                nc.gpsimd.local_scatter(
                    out_ap=Dq[:], data_ap=t0["SB"][:, cs], idxs_ap=t0["IV"][:, cs],
                    channels=P, num_elems=L, num_idxs=step,
                )
                parts.append(Dq)
            h = NI // 2
            Da = spool.tile([P, L], bf16, tag="d0", name="da_1")
            nc.gpsimd.local_scatter(
                out_ap=Da[:], data_ap=t1["SB"][:, :h], idxs_ap=t1["IV"][:, :h],
                channels=P, num_elems=L, num_idxs=h,
            )
            Db = spool.tile([P, L], bf16, tag="d1b", name="db_1")
            nc.gpsimd.local_scatter(
                out_ap=Db[:], data_ap=t1["SB"][:, h:], idxs_ap=t1["IV"][:, h:],
                channels=P, num_elems=L, num_idxs=NI - h,
            )

            # tile0 merge: pairwise bf16 adds (disjoint positions -> exact)
            M2a = spool.tile([P, L], bf16, tag="m2", name="m2a")
            nc.vector.tensor_tensor(out=M2a[:], in0=parts[0][:], in1=parts[1][:], op=Alu.add)
            M2b = spool.tile([P, L], bf16, tag="m2x", name="m2b")
            nc.vector.tensor_tensor(out=M2b[:], in0=parts[2][:], in1=parts[3][:], op=Alu.add)
            D0 = spool.tile([P, L], f32, tag="d1", name="d1_0")
            nc.vector.tensor_tensor(out=D0[:], in0=M2a[:], in1=M2b[:], op=Alu.add)

            # Act: all exps first (one Exp table load), lns at the end
            E0 = spool.tile([P, L], f32, tag="e", name="e0")
            nc.scalar.activation(out=E0[:], in_=D0[:], func=Act.Exp)
            Ea = spool.tile([P, L], f32, tag="ea", name="ea1")
            nc.scalar.activation(out=Ea[:], in_=Da[:], func=Act.Exp)
            Eb = spool.tile([P, L], f32, tag="eb", name="eb1")
            nc.scalar.activation(out=Eb[:], in_=Db[:], func=Act.Exp)

            # tile0 chain
            EM0 = spool.tile([P, L], f32, tag="em", name="em0")
            nc.vector.scalar_tensor_tensor(
                out=EM0[:], in0=IOTA[:], scalar=t0["LO"][:], in1=E0[:],
                op0=Alu.is_ge, op1=Alu.mult,
            )
            S0 = spool.tile([P, L], f32, tag="s", name="s0")
            nc.vector.tensor_tensor_scan(
                out=S0[:], data0=EM0[:], data1=EM0[:], initial=0.0,
                op0=Alu.add, op1=Alu.bypass,
            )
            nc.vector.scalar_tensor_tensor(
                out=SCR[:], in0=IOTA[:], scalar=t0["LO"][:], in1=D0[:],
                op0=Alu.is_ge, op1=Alu.mult, accum_out=stats[:, 0:1],
            )
            # tile1 chain: first-half ops can crawl under the last scatter
            EMa = spool.tile([P, L], f32, tag="ema", name="ema1")
            nc.vector.scalar_tensor_tensor(
                out=EMa[:], in0=IOTA[:], scalar=t1["LO"][:], in1=Ea[:],
                op0=Alu.is_ge, op1=Alu.mult,
            )
            nc.vector.scalar_tensor_tensor(
                out=SCR[:], in0=IOTA[:], scalar=t1["LO"][:], in1=Da[:],
                op0=Alu.is_ge, op1=Alu.mult, accum_out=stats[:, 2:3],
            )
            nc.vector.scalar_tensor_tensor(
                out=SCR[:], in0=IOTA[:], scalar=t1["LO"][:], in1=Db[:],
                op0=Alu.is_ge, op1=Alu.mult, accum_out=stats[:, 3:4],
            )
            EM1 = spool.tile([P, L], f32, tag="em1", name="em1")
            nc.vector.tensor_tensor(out=EM1[:], in0=EMa[:], in1=Eb[:], op=Alu.mult)
            S1 = spool.tile([P, L], f32, tag="s1", name="s1")
            nc.vector.tensor_tensor_scan(
                out=S1[:], data0=EM1[:], data1=EM1[:], initial=0.0,
                op0=Alu.add, op1=Alu.bypass,
            )
            # lns + masked lnS sums
            LN0 = spool.tile([P, L], f32, tag="ln", name="ln0")
            nc.scalar.activation(out=LN0[:], in_=S0[:], func=Act.Ln, bias=epsb[:], scale=1.0)
            nc.vector.scalar_tensor_tensor(
                out=SCR[:], in0=IOTA[:], scalar=t0["LO"][:], in1=LN0[:],
                op0=Alu.is_ge, op1=Alu.mult, accum_out=stats[:, 1:2],
            )
            LN1 = spool.tile([P, L], f32, tag="ln1", name="ln1")
            nc.scalar.activation(out=LN1[:], in_=S1[:], func=Act.Ln, bias=epsb[:], scale=1.0)
            nc.vector.scalar_tensor_tensor(
                out=SCR[:], in0=IOTA[:], scalar=t1["LO"][:], in1=LN1[:],
                op0=Alu.is_ge, op1=Alu.mult, accum_out=stats[:, 4:5],
            )
            nc.sync.dma_start(out=out[:], in_=stats[:])

    nc.compile()
    return nc


def _get_nc(K2):
    if K2 not in _cache:
        _cache[K2] = _build(K2)
    return _cache[K2]


def _host_prep(y_pred_scores, y_true_seqs):
    import ml_dtypes

    sc_b = np.ascontiguousarray(y_pred_scores.astype(ml_dtypes.bfloat16))
    seqs = np.asarray(y_true_seqs)
    rev = seqs[:, ::-1].astype(np.int32)  # pads (-1) now at the start
    npads = (seqs == -1).sum(1).astype(np.int32)

    # inverse mapping: INV[r, c] = smallest position l with rev[r, l] == c.
    # Assign positions from the back so the smallest l wins.
    INV = np.full(B * N, -1, np.int16)
    rowbase = np.arange(B, dtype=np.int64) * N
    for l in range(L - 1, -1, -1):
        c = rev[:, l]
        valid = c >= 0
        INV[rowbase[valid] + c[valid]] = l
    INV = INV.reshape(B, N)

    # extra occurrences (duplicated columns): positions whose column maps
    # to an earlier position
    ll = np.arange(L, dtype=np.int16)[None, :]
    first_of_col = np.where(rev >= 0, INV[np.arange(B)[:, None], np.clip(rev, 0, N - 1)], -1)
    extra = (rev >= 0) & (first_of_col != ll)
    counts = extra.sum(1)
    K2 = max(4, int(-(-int(counts.max()) // 4) * 4))
    fixi = np.full((B, K2), -1, np.int16)
    fixv = np.zeros((B, K2), ml_dtypes.bfloat16)
    er, el = np.nonzero(extra)
    # position of each extra within its row (0,1,2,...)
    k = np.zeros(len(er), np.int64)
    if len(er):
        newrow = np.r_[True, er[1:] != er[:-1]]
        idx = np.arange(len(er))
        k = idx - np.maximum.accumulate(np.where(newrow, idx, 0))
    fixi[er, k] = el.astype(np.int16)
    fixv[er, k] = sc_b[er, rev[er, el]]

    lo = (npads + 1).astype(np.float32).reshape(B, 1)
    used = npads < L
    data = np.ascontiguousarray(np.concatenate([sc_b, fixv], axis=1))
    idxs = np.ascontiguousarray(np.concatenate([INV, fixi], axis=1))
    return data, idxs, lo, used, K2


def kernel(y_pred_scores: np.ndarray, y_true_seqs: np.ndarray) -> np.ndarray:
    global LAST_RESULTS
    from concourse.bass_utils import run_bass_kernel_spmd

    data, idxs, lo, used, K2 = _host_prep(y_pred_scores, y_true_seqs)
    nc = _get_nc(K2)

    in_maps = []
    for c in range(NCORES):
        sl = slice(c * BL, (c + 1) * BL)
        in_maps.append(
            {
                "sc": data[sl],
                "inv": idxs[sl],
                "lo": lo[sl],
            }
        )

    res = run_bass_kernel_spmd(nc, in_maps, list(range(NCORES)), trace=TRACE)
    LAST_RESULTS = res

    n_used = int(used.sum())
    total_ll = 0.0
    for c in range(NCORES):
        st = res.results[c]["out"].astype(np.float64)  # [P, 5]
        r0 = slice(c * BL, c * BL + P)
        r1 = slice(c * BL + P, c * BL + 2 * P)
        total_ll += np.where(used[r0], st[:, 0] - st[:, 1], 0.0).sum()
        total_ll += np.where(
            used[r1], st[:, 2] + st[:, 3] - st[:, 4], 0.0
        ).sum()

    if n_used > 0:
        return np.float32(-total_ll / n_used)
    return np.float32(0.0)


# revision 14
# speedup vs baseline: 1.0194x; 1.0194x over previous
"""ListNet loss Trainium2 kernel.

kernel(y_pred_scores [2048, 8192] f32, y_true_seqs [2048, 512] int) -> () f32

Strategy: pure data parallel over the batch dim across 8 NeuronCores
(256 rows/core, 2 tiles of 128 rows). The per-row gather
g[p, l] = scores[p, seq[p, l]] is INVERTED into GPSIMD local_scatter,
the only on-chip primitive with per-partition independent indices:

  - host computes inv[p, c] = the sequence position (in reversed order)
    of column c's first occurrence, or -1 (ignored). Then
    local_scatter(data=scores_bf16[p, :], idxs=inv[p, :]) writes
    dst[p, inv[p, c]] = scores[p, c] -- the whole 512-wide gathered row
    in one pass over the natural score layout. ap_gather (shared index
    list per 16 partitions) would waste 15/16 of its output and is
    ~8x slower for this shape (~380us/core measured),
  - duplicated sequence indices (a column drawn at several positions)
    are appended as extra (value, position) columns to the data/idx
    arrays, so one scatter covers every occurrence,
  - each tile's scatter is split into column chunks (4 for tile 0, 2 for
    tile 1) so the first chunk starts as soon as its DMA slice lands;
    chunks write disjoint dst positions and are merged with adds. The
    whole kernel is jointly bound by HBM DMA (~6.2 MB/core) and the
    GPSIMD scatter stream (~25 us/tile), which overlap,
  - sequences are pre-reversed on host so pads sit at positions
    l < npads[row] and a forward prefix-sum scan of exp values yields
    the suffix softmax denominators S; the valid range [npads, 512) is
    selected with a device-built iota ramp >= (npads+1) per-partition
    mask,
  - LN = ln(S + eps); masked accumulating reductions give per-row
    sumg = sum of valid gathered scores, sumln = sum of valid LN.
Host: row_ll = sumg - sumln; used rows and the final mean in f64.

Scores are N(0,1) (sanitize is an identity on this data), so exp needs
no max-shift. bf16 score rounding (the scatter payload is 2-byte) gives
~2e-6 relative error on the final loss, far inside the 2e-2 gate.
"""

import numpy as np

B, N, L = 2048, 8192, 512
NCORES = 8
BL = B // NCORES  # 256 rows per core
P = 128
NT = BL // P  # tiles of 128 rows per core
EPS = 2.0**-126

TRACE = False
LAST_RESULTS = None

_cache = {}


def _build(K2):
    import concourse.bacc as bacc
    import concourse.mybir as mybir
    import concourse.tile as tile

    f32 = mybir.dt.float32
    bf16 = mybir.dt.bfloat16
    i16 = mybir.dt.int16
    Alu = mybir.AluOpType
    Act = mybir.ActivationFunctionType

    nc = bacc.Bacc("TRN2", target_bir_lowering=False, debug=False)
    NI = N + K2  # score columns + appended duplicate-fix entries
    sc = nc.dram_tensor("sc", [BL, NI], bf16, kind="ExternalInput").ap()
    inv = nc.dram_tensor("inv", [BL, NI], i16, kind="ExternalInput").ap()
    lo = nc.dram_tensor("lo", [BL, 1], f32, kind="ExternalInput").ap()
    # out columns per tile t: [sumg, sumln]
    out = nc.dram_tensor("out", [P, 2 * NT], f32, kind="ExternalOutput").ap()

    with tile.TileContext(nc) as tc:
        with (
            tc.tile_pool(name="const", bufs=1) as cpool,
            tc.tile_pool(name="big", bufs=2) as bpool,
            tc.tile_pool(name="small", bufs=2) as spool,
        ):
            IOTA = cpool.tile([P, L], f32)
            epsb = cpool.tile([P, 1], f32)
            nc.vector.memset(epsb[:], EPS)
            stats = cpool.tile([P, 2 * NT], f32)
            SCR = cpool.tile([P, L], f32)  # dead-write target for accum ops

            tl = []  # per-tile dict of tiles
            for t in range(NT):
                rows = slice(t * P, (t + 1) * P)
                d = {}
                d["IV"] = bpool.tile([P, NI], i16, tag="iv", name=f"iv{t}")
                d["SB"] = bpool.tile([P, NI], bf16, tag="sb", name=f"sb{t}")
                # chunked DMA, alternating queues for byte balance; chunk
                # boundaries match the scatter splits below
                nch = 4 if t == 0 else 2
                step = NI // nch
                for q in range(nch):
                    cs = slice(q * step, (q + 1) * step)
                    qa = nc.sync if q % 2 == 0 else nc.scalar
                    qb = nc.scalar if q % 2 == 0 else nc.sync
                    qa.dma_start(out=d["IV"][:, cs], in_=inv[rows, cs])
                    qb.dma_start(out=d["SB"][:, cs], in_=sc[rows, cs])
                d["LO"] = spool.tile([P, 1], f32, tag="lo", name=f"lo{t}")
                tl.append(d)
            for t in range(NT):
                nc.scalar.dma_start(out=tl[t]["LO"][:], in_=lo[slice(t * P, (t + 1) * P), :])
            # IOTA = [1, 2, ..., L] built on device: prefix-scan of ones
            # (host sends npads+1 as the mask bound)
            nc.vector.memset(SCR[:], 1.0)
            nc.vector.tensor_tensor_scan(
                out=IOTA[:],
                data0=SCR[:],
                data1=SCR[:],
                initial=0.0,
                op0=Alu.add,
                op1=Alu.bypass,
            )
            # gpsimd: chunked column scatters per tile (the serial resource);
            # chunks write disjoint dst positions, merged with adds
            for t in range(NT):
                d = tl[t]
                nch = 4 if t == 0 else 2
                step = NI // nch
                parts = []
                for q in range(nch):
                    cs = slice(q * step, (q + 1) * step)
                    Dq = spool.tile(
                        [P, L], bf16, tag=f"d{q}", name=f"d{q}_{t}"
                    )
                    nc.gpsimd.local_scatter(
                        out_ap=Dq[:],
                        data_ap=d["SB"][:, cs],
                        idxs_ap=d["IV"][:, cs],
                        channels=P,
                        num_elems=L,
                        num_idxs=step,
                    )
                    parts.append(Dq)
                # pairwise merge (bf16 + bf16 -> f32 at the last add)
                while len(parts) > 2:
                    a = parts.pop(0)
                    b = parts.pop(0)
                    M2 = spool.tile(
                        [P, L], bf16, tag="m2", name=f"m2_{t}_{len(parts)}"
                    )
                    nc.vector.tensor_tensor(
                        out=M2[:], in0=a[:], in1=b[:], op=Alu.add
                    )
                    parts.append(M2)
                d["D1"] = spool.tile([P, L], f32, tag="d1", name=f"d1_{t}")
                nc.vector.tensor_tensor(
                    out=d["D1"][:], in0=parts[0][:], in1=parts[1][:], op=Alu.add
                )
                d["E"] = spool.tile([P, L], f32, tag="e", name=f"e{t}")
                nc.scalar.activation(out=d["E"][:], in_=d["D1"][:], func=Act.Exp)
            # per-tile compute chains (all small: [128, 512])
            for t in range(NT):
                d = tl[t]
                d["EM"] = spool.tile([P, L], f32, tag="em", name=f"em{t}")
                nc.vector.scalar_tensor_tensor(
                    out=d["EM"][:],
                    in0=IOTA[:],
                    scalar=d["LO"][:],
                    in1=d["E"][:],
                    op0=Alu.is_ge,
                    op1=Alu.mult,
                )
                d["S"] = spool.tile([P, L], f32, tag="s", name=f"s{t}")
                nc.vector.tensor_tensor_scan(
                    out=d["S"][:],
                    data0=d["EM"][:],
                    data1=d["EM"][:],
                    initial=0.0,
                    op0=Alu.add,
                    op1=Alu.bypass,
                )
                nc.vector.scalar_tensor_tensor(
                    out=SCR[:],
                    in0=IOTA[:],
                    scalar=d["LO"][:],
                    in1=d["D1"][:],
                    op0=Alu.is_ge,
                    op1=Alu.mult,
                    accum_out=stats[:, 2 * t : 2 * t + 1],
                )
            for t in range(NT):
                d = tl[t]
                d["LN"] = spool.tile([P, L], f32, tag="ln", name=f"ln{t}")
                nc.scalar.activation(
                    out=d["LN"][:], in_=d["S"][:], func=Act.Ln, bias=epsb[:], scale=1.0
                )
                nc.vector.scalar_tensor_tensor(
                    out=SCR[:],
                    in0=IOTA[:],
                    scalar=d["LO"][:],
                    in1=d["LN"][:],
                    op0=Alu.is_ge,
                    op1=Alu.mult,
                    accum_out=stats[:, 2 * t + 1 : 2 * t + 2],
                )
            nc.sync.dma_start(out=out[:], in_=stats[:])

    nc.compile()
    return nc


def _get_nc(K2):
    if K2 not in _cache:
        _cache[K2] = _build(K2)
    return _cache[K2]


def _host_prep(y_pred_scores, y_true_seqs):
    import ml_dtypes

    sc_b = np.ascontiguousarray(y_pred_scores.astype(ml_dtypes.bfloat16))
    seqs = np.asarray(y_true_seqs)
    rev = seqs[:, ::-1].astype(np.int32)  # pads (-1) now at the start
    npads = (seqs == -1).sum(1).astype(np.int32)

    # inverse mapping: INV[r, c] = smallest position l with rev[r, l] == c.
    # Assign positions from the back so the smallest l wins.
    INV = np.full(B * N, -1, np.int16)
    rowbase = np.arange(B, dtype=np.int64) * N
    for l in range(L - 1, -1, -1):
        c = rev[:, l]
        valid = c >= 0
        INV[rowbase[valid] + c[valid]] = l
    INV = INV.reshape(B, N)

    # extra occurrences (duplicated columns): positions whose column maps
    # to an earlier position
    ll = np.arange(L, dtype=np.int16)[None, :]
    first_of_col = np.where(rev >= 0, INV[np.arange(B)[:, None], np.clip(rev, 0, N - 1)], -1)
    extra = (rev >= 0) & (first_of_col != ll)
    counts = extra.sum(1)
    K2 = max(4, int(-(-int(counts.max()) // 4) * 4))
    fixi = np.full((B, K2), -1, np.int16)
    fixv = np.zeros((B, K2), ml_dtypes.bfloat16)
    er, el = np.nonzero(extra)
    # position of each extra within its row (0,1,2,...)
    k = np.zeros(len(er), np.int64)
    if len(er):
        newrow = np.r_[True, er[1:] != er[:-1]]
        idx = np.arange(len(er))
        k = idx - np.maximum.accumulate(np.where(newrow, idx, 0))
    fixi[er, k] = el.astype(np.int16)
    fixv[er, k] = sc_b[er, rev[er, el]]

    lo = (npads + 1).astype(np.float32).reshape(B, 1)
    used = npads < L
    data = np.ascontiguousarray(np.concatenate([sc_b, fixv], axis=1))
    idxs = np.ascontiguousarray(np.concatenate([INV, fixi], axis=1))
    return data, idxs, lo, used, K2


def kernel(y_pred_scores: np.ndarray, y_true_seqs: np.ndarray) -> np.ndarray:
    global LAST_RESULTS
    from concourse.bass_utils import run_bass_kernel_spmd

    data, idxs, lo, used, K2 = _host_prep(y_pred_scores, y_true_seqs)
    nc = _get_nc(K2)

    in_maps = []
    for c in range(NCORES):
        sl = slice(c * BL, (c + 1) * BL)
        in_maps.append(
            {
                "sc": data[sl],
                "inv": idxs[sl],
                "lo": lo[sl],
            }
        )

    res = run_bass_kernel_spmd(nc, in_maps, list(range(NCORES)), trace=TRACE)
    LAST_RESULTS = res

    n_used = int(used.sum())
    total_ll = 0.0
    for c in range(NCORES):
        st = res.results[c]["out"].astype(np.float64)  # [P, 2*NT]
        for t in range(NT):
            rows = slice(c * BL + t * P, c * BL + (t + 1) * P)
            row_ll = st[:, 2 * t] - st[:, 2 * t + 1]
            total_ll += np.where(used[rows], row_ll, 0.0).sum()

    if n_used > 0:
        return np.float32(-total_ll / n_used)
    return np.float32(0.0)
